# revision 24
# baseline (speedup 1.0000x reference)
"""LocalSelfAttention (window=7) Trainium2 Bass kernel.

Full inputs in, full output out. Sharding: 8 cores = batch(4) x seq-half(2),
each core handles 1024 tokens with a 3-token zero-padded halo on xs.

Math notes (exact rewrites of the reference):
- reference projects zero-PADDED xs patches, so out-of-range taps have
  k = b_ks, v = b_vs. Softmax over taps is invariant to the per-(t,h)
  constant q . b_ks, so the K bias drops entirely (padded taps then score 0,
  matching zero-padded halo @ w_ks with no bias).
- softmax weights sum to 1, so the V bias contributes exactly b_vs to o;
  b' = b_vs @ w_fc + b_fc is folded into the residual on the host:
  xqT carries (x + b')^T and b_qs is corrected by -b' @ w_qs.
- 1/TEMP is folded into w_qs/b_qs on the host.

Pipeline per core (feature-major activations, transposed on the HOST):
- QT/KT feature-major via matmul(lhsT=weight tile, rhs=xT), V token-major.
- attention in 11 chunks of 96 tokens, head PAIRS: band mask preloaded
  into PSUM by a PE matmul (identity x mask, start=True), score matmuls
  accumulate on top (start=False); ACT Exp reads PSUM directly -> bf16
  probs in SBUF + per-query row sums via accum_out; DVE reciprocal; the
  prob transpose is a matmul by diag(1/Z) (normalization fused into the
  transpose); PV matmuls accumulate both heads into one (128, 96) PSUM.
- FC consumes OT; the residual x rides the same PSUM accumulation via
  identity matmuls from xT; LayerNorm runs directly off PSUM (ACT Square
  accum for sumsq, DVE reduce for sum, ACT Identity scale/bias evict).
- V projection / attention chunks / FC+LN are interleaved in emission
  order to keep the PE instruction stream dense (p-state at 2.4 GHz).
"""

import sys

for _p in ("/opt/trn_rl_repo",):
    if _p not in sys.path:
        sys.path.insert(0, _p)

import numpy as np
import ml_dtypes

BF16 = ml_dtypes.bfloat16

H, DK, DV, D = 16, 64, 64, 1024
NEI = 3
TEMP = 8.0
EPS = 1e-5
B, S = 4, 2048
NCORES = 8
T = (B * S) // NCORES          # 1024 tokens per core
TH = T + 2 * NEI               # 1030 halo tokens
P = 128
NT = T // P                    # 8 fc-phase token chunks
ND = D // P                    # 8 feature chunks
CL = 96                        # attention chunk length
CST = [96 * i for i in range(10)] + [928]          # chunk starts
NCH = len(CST)                 # 11 attention chunks
TH2 = 1056                     # padded halo width (window reads up to 1056)
NEG = -30000.0

_CACHE = {}


def _build_program(apply_affine: bool):
    import concourse.bacc as bacc
    import concourse.tile as tile
    from concourse import mybir
    from contextlib import ExitStack

    # Pin all activations to the one table set that contains every function
    # we use (Exp, Ln, Identity, Copy, Square). The default first-fit pass
    # alternates between exp- and ln- sets, costing a 1.3us table reload per
    # FC chunk. Indices into act_info.json are preserved; only the candidate
    # function sets are restricted. Restored after compile.
    _orig_tables = bacc.get_activation_tables

    def _pinned_tables(arch):
        t = _orig_tables(arch)
        keep = "natural_log_exp_and_others"
        if keep not in t:
            return t
        return {name: (funcs if name == keep else frozenset())
                for name, funcs in t.items()}

    f32 = mybir.dt.float32
    bf16 = mybir.dt.bfloat16
    Alu = mybir.AluOpType
    Act = mybir.ActivationFunctionType

    bacc.get_activation_tables = _pinned_tables
    nc = bacc.Bacc(
        "TRN2", target_bir_lowering=False, debug=False, enable_asserts=False
    )

    def din(name, shape, dt_):
        return nc.dram_tensor(name, shape, dt_, kind="ExternalInput").ap()

    xqT = din("xqT", (D, T), bf16)       # (x + b')^T (host-transposed)
    xsT = din("xsT", (D, TH), bf16)      # xs^T with halo (host-transposed)
    wq = din("wq", (D, D), bf16)         # w_qs / TEMP
    wk = din("wk", (D, D), bf16)
    wv = din("wv", (D, D), bf16)
    wf = din("wf", (D, D), bf16)
    bq = din("bq", (P, ND), f32)         # (b_qs - b'@w_qs)/TEMP laid out [p, ec]
    msk = din("msk", (CL, P), bf16)      # band mask 0 / NEG
    id96 = din("id96", (CL, CL), bf16)   # identity for mask preload + transpose
    id128 = din("id128", (P, P), bf16)   # identity for residual matmul
    if apply_affine:
        lng = din("lng", (1, D), f32)
        lnb = din("lnb", (1, D), f32)
    yo = nc.dram_tensor("yo", (T, D), f32, kind="ExternalOutput").ap()

    with tile.TileContext(nc) as tc, ExitStack() as ctx:
        consts = ctx.enter_context(tc.tile_pool(name="consts", bufs=1))
        big = ctx.enter_context(tc.tile_pool(name="big", bufs=1))
        wpool = ctx.enter_context(tc.tile_pool(name="wpool", bufs=2))
        work = ctx.enter_context(tc.tile_pool(name="work", bufs=3))
        opool = ctx.enter_context(tc.tile_pool(name="opool", bufs=2))
        small = ctx.enter_context(tc.tile_pool(name="small", bufs=4))
        ps1 = ctx.enter_context(tc.tile_pool(name="ps1", bufs=4, space="PSUM"))
        psT = ctx.enter_context(tc.tile_pool(name="psT", bufs=1, space="PSUM"))
        psF = ctx.enter_context(tc.tile_pool(name="psF", bufs=1, space="PSUM"))

        # ---- weights first (Q-proj critical path), then consts ----
        def load_w(wap, tagp):
            tiles = []
            for dc in range(ND):
                wt = wpool.tile([P, D], bf16, tag=f"w{dc}", name=f"w_{tagp}{dc}")
                nc.sync.dma_start(out=wt, in_=wap[dc * P:(dc + 1) * P, :])
                tiles.append(wt)
            return tiles

        wq_t = load_w(wq, "q")
        xT_t = []
        for dc in range(ND):
            t1 = big.tile([P, T], bf16, tag=f"xT{dc}", name=f"xT{dc}")
            nc.sync.dma_start(out=t1, in_=xqT[dc * P:(dc + 1) * P, :])
            xT_t.append(t1)
        wk_t = load_w(wk, "k")
        xsT_t = []
        for dc in range(ND):
            t2 = big.tile([P, TH2], bf16, tag=f"xsT{dc}", name=f"xsT{dc}")
            nc.sync.dma_start(out=t2[:, 0:TH], in_=xsT[dc * P:(dc + 1) * P, :])
            nc.vector.memset(t2[:, TH:TH2], 0.0)
            xsT_t.append(t2)

        bq_sb = consts.tile([P, ND], f32, tag="bq")
        nc.sync.dma_start(out=bq_sb, in_=bq)
        msk_sb = consts.tile([CL, P], bf16, tag="msk")
        nc.sync.dma_start(out=msk_sb, in_=msk)
        id96_sb = consts.tile([CL, CL], bf16, tag="id96")
        nc.sync.dma_start(out=id96_sb, in_=id96)
        id128_sb = consts.tile([P, P], bf16, tag="id128")
        nc.sync.dma_start(out=id128_sb, in_=id128)
        eps_sb = consts.tile([P, 1], f32, tag="eps")
        nc.vector.memset(eps_sb, EPS)
        if apply_affine:
            import concourse.bass as bass

            g_bc = consts.tile([P, D], f32, tag="g_bc")
            b_bc = consts.tile([P, D], f32, tag="b_bc")
            nc.sync.dma_start(
                out=g_bc,
                in_=bass.AP(tensor=lng.tensor, offset=lng.offset,
                            ap=[[0, P]] + list(lng.ap[1:])),
            )
            nc.sync.dma_start(
                out=b_bc,
                in_=bass.AP(tensor=lnb.tensor, offset=lnb.offset,
                            ap=[[0, P]] + list(lnb.ap[1:])),
            )

        # ---- QT projection: (e, t) feature-major, bias via ACT evict ----
        QT = [big.tile([P, T], bf16, tag=f"QT{ec}", name=f"QT{ec}")
              for ec in range(ND)]
        for ec in range(ND):
            psa = ps1.tile([P, 512], f32, tag="ps1", name="ps_qa")
            psb = ps1.tile([P, 512], f32, tag="ps1", name="ps_qb")
            for dc in range(ND):
                lt = wq_t[dc][:, ec * P:(ec + 1) * P]
                nc.tensor.matmul(psa, lhsT=lt, rhs=xT_t[dc][:, 0:512],
                                 start=(dc == 0), stop=(dc == ND - 1))
                nc.tensor.matmul(psb, lhsT=lt, rhs=xT_t[dc][:, 512:1024],
                                 start=(dc == 0), stop=(dc == ND - 1))
            nc.scalar.activation(out=QT[ec][:, 0:512], in_=psa,
                                 func=Act.Identity,
                                 bias=bq_sb[:, ec:ec + 1], scale=1.0)
            nc.scalar.activation(out=QT[ec][:, 512:1024], in_=psb,
                                 func=Act.Identity,
                                 bias=bq_sb[:, ec:ec + 1], scale=1.0)

        # ---- KT projection: (e, t_halo) feature-major, no bias ----
        KT = [big.tile([P, TH2], bf16, tag=f"KT{ec}", name=f"KT{ec}")
              for ec in range(ND)]
        for ec in range(ND):
            psa = ps1.tile([P, 512], f32, tag="ps1", name="ps_ka")
            psb = ps1.tile([P, 512], f32, tag="ps1", name="ps_kb")
            for dc in range(ND):
                lt = wk_t[dc][:, ec * P:(ec + 1) * P]
                nc.tensor.matmul(psa, lhsT=lt, rhs=xsT_t[dc][:, 0:512],
                                 start=(dc == 0), stop=(dc == ND - 1))
                nc.tensor.matmul(psb, lhsT=lt, rhs=xsT_t[dc][:, 512:1024],
                                 start=(dc == 0), stop=(dc == ND - 1))
            nc.vector.tensor_copy(KT[ec][:, 0:512], psa)
            nc.vector.tensor_copy(KT[ec][:, 512:1024], psb)
        for ec in range(ND):  # halo tail (incl zero padding)
            pst = ps1.tile([P, 512], f32, tag="ps1", name="ps_kt")
            for dc in range(ND):
                nc.tensor.matmul(pst[:, 0:TH2 - T],
                                 lhsT=wk_t[dc][:, ec * P:(ec + 1) * P],
                                 rhs=xsT_t[dc][:, T:TH2],
                                 start=(dc == 0), stop=(dc == ND - 1))
            nc.vector.tensor_copy(KT[ec][:, T:TH2], pst[:, 0:TH2 - T])

        # weight prefetch for the fused phase
        wv_t = load_w(wv, "v")
        wf_t = load_w(wf, "f")

        V = [None] * NCH
        OT = [big.tile([P, T], bf16, tag=f"OT{ec}", name=f"OT{ec}")
              for ec in range(ND)]

        # ---------- fused-phase emission helpers ----------
        def emit_v(ci):
            s = CST[ci]
            vt = big.tile([P, D], bf16, tag=f"V{ci}", name=f"V{ci}")
            psa = ps1.tile([P, 512], f32, tag="ps1", name="ps_va")
            psb = ps1.tile([P, 512], f32, tag="ps1", name="ps_vb")
            for dc in range(ND):
                lt = xsT_t[dc][:, s:s + P]
                nc.tensor.matmul(psa, lhsT=lt, rhs=wv_t[dc][:, 0:512],
                                 start=(dc == 0), stop=(dc == ND - 1))
                nc.tensor.matmul(psb, lhsT=lt, rhs=wv_t[dc][:, 512:1024],
                                 start=(dc == 0), stop=(dc == ND - 1))
            nc.vector.tensor_copy(vt[:, 0:512], psa)
            nc.vector.tensor_copy(vt[:, 512:1024], psb)
            V[ci] = vt

        # persistent normalized-prob tiles: live key cols [0:LW); the dead
        # tail [LW:P) is zeroed once so transposes can read a full 128 keys
        LW = 104  # live key window per 96-query chunk (96 + 6, padded to x8)
        pn_tiles = []
        for j in range(3):
            pnt = big.tile([CL, 2 * P], bf16, tag=f"pn{j}", name=f"pn{j}")
            nc.vector.memset(
                pnt.rearrange("p (h w) -> p h w", h=2)[:, :, LW:P], 0.0)
            pn_tiles.append(pnt)

        def pair_head(idx, ci, ec):
            s = CST[ci]
            # scores + band mask accumulated in two PSUM banks (one/head)
            sa = ps1.tile([P, 512], f32, tag="ps1", name="sa")
            sb = ps1.tile([P, 512], f32, tag="ps1", name="sb")
            nc.tensor.matmul(sa[0:CL, 0:LW], lhsT=id96_sb,
                             rhs=msk_sb[:, 0:LW], start=True, stop=False)
            nc.tensor.matmul(sa[0:CL, 0:LW],
                             lhsT=QT[ec][0:64, s:s + CL],
                             rhs=KT[ec][0:64, s:s + LW],
                             start=False, stop=True)
            nc.tensor.matmul(sb[0:CL, 0:LW], lhsT=id96_sb,
                             rhs=msk_sb[:, 0:LW], start=True, stop=False)
            nc.tensor.matmul(sb[0:CL, 0:LW],
                             lhsT=QT[ec][64:128, s:s + CL],
                             rhs=KT[ec][64:128, s:s + LW],
                             start=False, stop=True)
            return {"idx": idx, "ci": ci, "ec": ec, "sa": sa, "sb": sb}

        def pair_mid(st):
            pe2 = work.tile([CL, 2 * LW], bf16, tag="pe2", name="pe2")
            nc.scalar.activation(out=pe2[:, 0:LW],
                                 in_=st["sa"][0:CL, 0:LW], func=Act.Exp)
            nc.scalar.activation(out=pe2[:, LW:2 * LW],
                                 in_=st["sb"][0:CL, 0:LW], func=Act.Exp)
            rs2 = small.tile([CL, 2], f32, tag="rs2", name="rs2")
            nc.vector.tensor_reduce(
                out=rs2, in_=pe2.rearrange("p (h w) -> p h w", h=2),
                axis=mybir.AxisListType.X, op=Alu.add)
            rsr2 = small.tile([CL, 2], f32, tag="rsr2", name="rsr2")
            nc.vector.reciprocal(rsr2, rs2)
            pn2 = pn_tiles[st["idx"] % 3]
            nc.gpsimd.tensor_tensor(
                pn2.rearrange("p (h w) -> p h w", h=2)[:, :, 0:LW],
                pe2.rearrange("p (h w) -> p h w", h=2),
                rsr2[:, :, None].to_broadcast((CL, 2, LW)),
                Alu.mult,
            )
            st["pn2"] = pn2

        def pair_tail(st):
            ci, ec, pn2 = st["ci"], st["ec"], st["pn2"]
            s = CST[ci]
            pt = psT.tile([P, 2048], bf16, tag="psT", name="pt")
            nc.tensor.transpose(pt[:, 0:CL], pn2[:, 0:P], id96_sb)
            nc.tensor.transpose(pt[:, 1024:1024 + CL], pn2[:, P:2 * P],
                                id96_sb)
            ptsb = work.tile([P, 2 * CL], bf16, tag="ptsb", name="ptsb")
            nc.vector.tensor_copy(
                ptsb.rearrange("p (h w) -> p h w", h=2),
                pt.rearrange("p (h w) -> p h w", h=2)[:, 0:2, 0:CL],
            )
            ot2 = ps1.tile([P, 512], f32, tag="ps1", name="ot2")
            nc.tensor.matmul(ot2[0:64, 0:CL],
                             lhsT=V[ci][:, ec * P:ec * P + 64],
                             rhs=ptsb[:, 0:CL], start=True, stop=True)
            nc.tensor.matmul(ot2[64:128, 0:CL],
                             lhsT=V[ci][:, ec * P + 64:(ec + 1) * P],
                             rhs=ptsb[:, CL:2 * CL], start=True, stop=True)
            nc.scalar.activation(out=OT[ec][:, s:s + CL], in_=ot2[:, 0:CL],
                                 func=Act.Copy)

        def emit_fc(c):
            cs = slice(c * P, (c + 1) * P)
            fa = psF.tile([P, 512], f32, tag="fA", name="fa")
            fb = psF.tile([P, 512], f32, tag="fB", name="fb")
            # one full-region start per bank; residual (x + b') identity
            # matmuls accumulate into 128-col subregions of the open group
            nc.tensor.matmul(fa, lhsT=OT[0][:, cs], rhs=wf_t[0][:, 0:512],
                             start=True, stop=False)
            nc.tensor.matmul(fb, lhsT=OT[0][:, cs], rhs=wf_t[0][:, 512:1024],
                             start=True, stop=False)
            for db in range(4):
                nc.tensor.matmul(fa[:, db * P:(db + 1) * P],
                                 lhsT=xT_t[db][:, cs], rhs=id128_sb,
                                 start=False, stop=False)
                nc.tensor.matmul(fb[:, db * P:(db + 1) * P],
                                 lhsT=xT_t[4 + db][:, cs], rhs=id128_sb,
                                 start=False, stop=False)
            for ec in range(1, ND):
                lt = OT[ec][:, cs]
                nc.tensor.matmul(fa, lhsT=lt, rhs=wf_t[ec][:, 0:512],
                                 start=False, stop=(ec == ND - 1))
                nc.tensor.matmul(fb, lhsT=lt, rhs=wf_t[ec][:, 512:1024],
                                 start=False, stop=(ec == ND - 1))
            # evict y to SBUF bf16 IMMEDIATELY to release the psF banks
            # (keeping LN reads on PSUM stalls the next chunk's FC matmuls)
            y_sb = opool.tile([P, D], bf16, tag="ysb", name="y_sb")
            ysA = small.tile([P, 1], f32, tag="ysA", name="ysA")
            ysB = small.tile([P, 1], f32, tag="ysB", name="ysB")
            nc.scalar.activation(out=y_sb[:, 0:512], in_=fa, func=Act.Copy)
            nc.vector.tensor_copy(y_sb[:, 512:1024], fb)
            nc.vector.tensor_reduce(out=ysA, in_=y_sb[:, 0:512],
                                    axis=mybir.AxisListType.X, op=Alu.add)
            nc.vector.tensor_reduce(out=ysB, in_=y_sb[:, 512:1024],
                                    axis=mybir.AxisListType.X, op=Alu.add)
            ysq = opool.tile([P, D], bf16, tag="ysq", name="ysq")
            sqA = small.tile([P, 1], f32, tag="sqA", name="sqA")
            sqB = small.tile([P, 1], f32, tag="sqB", name="sqB")
            nc.vector.tensor_mul(ysq[:, 0:512], y_sb[:, 0:512],
                                 y_sb[:, 0:512])
            nc.vector.tensor_mul(ysq[:, 512:1024], y_sb[:, 512:1024],
                                 y_sb[:, 512:1024])
            nc.vector.tensor_reduce(out=sqA, in_=ysq[:, 0:512],
                                    axis=mybir.AxisListType.X, op=Alu.add)
            nc.vector.tensor_reduce(out=sqB, in_=ysq[:, 512:1024],
                                    axis=mybir.AxisListType.X, op=Alu.add)
            ysum = small.tile([P, 1], f32, tag="ysum", name="ysum")
            nc.vector.tensor_add(ysum, ysA, ysB)
            ssum = small.tile([P, 1], f32, tag="ssum", name="ssum")
            nc.vector.tensor_add(ssum, sqA, sqB)
            mean = small.tile([P, 1], f32, tag="mean", name="mean")
            nc.vector.tensor_scalar_mul(mean, ysum, 1.0 / D)
            msq = small.tile([P, 1], f32, tag="msq", name="msq")
            nc.vector.tensor_mul(msq, mean, mean)
            var = small.tile([P, 1], f32, tag="var", name="var")
            nc.vector.scalar_tensor_tensor(
                out=var, in0=ssum, scalar=1.0 / D, in1=msq,
                op0=Alu.mult, op1=Alu.subtract,
            )
            # rstd = exp(-0.5*ln(var+eps)): keeps ACT on one function table
            # (Sqrt lives in a different act set and forces 1.3us reloads)
            lnv = small.tile([P, 1], f32, tag="lnv", name="lnv")
            nc.scalar.activation(out=lnv, in_=var, func=Act.Ln, bias=eps_sb)
            rstd = small.tile([P, 1], f32, tag="rstd", name="rstd")
            nc.scalar.activation(out=rstd, in_=lnv, func=Act.Exp, scale=-0.5)
            bact = small.tile([P, 1], f32, tag="bact", name="bact")
            nc.vector.scalar_tensor_tensor(
                out=bact, in0=mean, scalar=-1.0, in1=rstd,
                op0=Alu.mult, op1=Alu.mult,
            )
            out_sb = opool.tile([P, D], f32, tag="osb", name="out_sb")
            nc.scalar.activation(out=out_sb[:, 0:512], in_=y_sb[:, 0:512],
                                 func=Act.Identity, bias=bact, scale=rstd)
            nc.scalar.activation(out=out_sb[:, 512:1024],
                                 in_=y_sb[:, 512:1024],
                                 func=Act.Identity, bias=bact, scale=rstd)
            if apply_affine:
                nc.vector.tensor_mul(out_sb, out_sb, g_bc)
                nc.vector.tensor_add(out_sb, out_sb, b_bc)
            nc.sync.dma_start(out=yo[cs, :], in_=out_sb)

        # FC chunk c is ready after attention chunk a_last(c)
        a_last = [((c + 1) * P - 1) // CL for c in range(NT)]
        a_last[NT - 1] = NCH - 1

        # ---------- fused phase: V / attention / FC interleaved ----------
        # 3-stage software pipeline over head pairs: emit pair i's scores
        # (PE) two pairs ahead of its transpose/PV (PE), so the PE stream
        # never head-of-line blocks on the cross-engine softmax chain.
        emit_v(0)
        emit_v(1)
        seq = [(ci, ec) for ci in range(NCH) for ec in range(ND)]
        stq = []
        done = 0

        def after_tail(st):
            # FC chunks become ready once the last pair of chunk ci retires
            if st["ec"] == ND - 1:
                for c in range(NT):
                    if a_last[c] == st["ci"]:
                        emit_fc(c)

        for idx, (ci, ec) in enumerate(seq):
            stq.append(pair_head(idx, ci, ec))
            if ec == 3 and ci + 2 < NCH:
                emit_v(ci + 2)
            if len(stq) - done >= 2:
                pair_mid(stq[-2])
            if len(stq) - done >= 3:
                pair_tail(stq[done])
                after_tail(stq[done])
                stq[done] = None
                done += 1
        pair_mid(stq[-1])
        for k in range(done, len(seq)):
            pair_tail(stq[k])
            after_tail(stq[k])
            stq[k] = None

    try:
        nc.compile()
    finally:
        bacc.get_activation_tables = _orig_tables
    return nc


def _get_program(apply_affine: bool):
    key = ("prog", apply_affine)
    if key not in _CACHE:
        _CACHE[key] = _build_program(apply_affine)
    return _CACHE[key]


def _host_prep(inputs):
    x = np.asarray(inputs["x"], np.float32)
    xs = np.asarray(inputs["xs"], np.float32)
    w_qs = np.asarray(inputs["w_qs"], np.float32)
    b_qs = np.asarray(inputs["b_qs"], np.float32)
    w_ks = np.asarray(inputs["w_ks"], np.float32)
    w_vs = np.asarray(inputs["w_vs"], np.float32)
    b_vs = np.asarray(inputs["b_vs"], np.float32)
    w_fc = np.asarray(inputs["w_fc"], np.float32)
    b_fc = np.asarray(inputs["b_fc"], np.float32)
    ln_g = np.asarray(inputs["ln_g"], np.float32)
    ln_b = np.asarray(inputs["ln_b"], np.float32)

    apply_affine = not (np.all(ln_g == 1.0) and np.all(ln_b == 0.0))

    bprime = (b_vs @ w_fc + b_fc).astype(np.float32)
    bq_eff = (b_qs - bprime @ w_qs) / TEMP

    mask = np.full((CL, P), NEG, np.float32)
    for t in range(CL):
        mask[t, t:t + 2 * NEI + 1] = 0.0

    shared = {
        "wq": np.ascontiguousarray((w_qs / TEMP).astype(BF16)),
        "wk": np.ascontiguousarray(w_ks.astype(BF16)),
        "wv": np.ascontiguousarray(w_vs.astype(BF16)),
        "wf": np.ascontiguousarray(w_fc.astype(BF16)),
        "bq": np.ascontiguousarray(bq_eff.reshape(ND, P).T.astype(np.float32)),
        "msk": np.ascontiguousarray(mask.astype(BF16)),
        "id96": np.eye(CL, dtype=BF16),
        "id128": np.eye(P, dtype=BF16),
    }
    if apply_affine:
        shared["lng"] = np.ascontiguousarray(ln_g.reshape(1, D))
        shared["lnb"] = np.ascontiguousarray(ln_b.reshape(1, D))

    xr = x + bprime[None, None, :]
    in_maps = []
    half_n = S // 2  # 1024
    for core in range(NCORES):
        b, half = core // 2, core % 2
        t0 = half * half_n
        halo = np.zeros((TH, D), np.float32)
        lo = max(0, t0 - NEI)
        hi = min(S, t0 + half_n + NEI)
        halo[lo - (t0 - NEI):hi - (t0 - NEI)] = xs[b, lo:hi]
        m = dict(shared)
        m["xqT"] = np.ascontiguousarray(xr[b, t0:t0 + half_n].T.astype(BF16))
        m["xsT"] = np.ascontiguousarray(halo.T.astype(BF16))
        in_maps.append(m)
    return in_maps, apply_affine


def _run(inputs, trace=False, trace_kwargs=None):
    from concourse.bass_utils import run_bass_kernel_spmd

    in_maps, apply_affine = _host_prep(inputs)
    nc = _get_program(apply_affine)
    res = run_bass_kernel_spmd(
        nc, in_maps, list(range(NCORES)),
        trace=trace, **(trace_kwargs or {})
    )
    y = np.empty((B, S, D), np.float32)
    half_n = S // 2
    for core in range(NCORES):
        b, half = core // 2, core % 2
        y[b, half * half_n:(half + 1) * half_n] = res.results[core]["yo"]
    return y, res


def kernel(**inputs):
    y, _ = _run(inputs)
    return y


# revision 26
# speedup vs baseline: 1.0621x; 1.0621x over previous
"""LocalSelfAttention (window=7) Trainium2 Bass kernel.

Full inputs in, full output out. Sharding: 8 cores = batch(4) x seq-half(2),
each core handles 1024 tokens with a 3-token zero-padded halo on xs.

Math notes (exact rewrites of the reference):
- reference projects zero-PADDED xs patches, so out-of-range taps have
  k = b_ks, v = b_vs. Softmax over taps is invariant to the per-(t,h)
  constant q . b_ks, so the K bias drops entirely (padded taps then score 0,
  matching zero-padded halo @ w_ks with no bias).
- softmax weights sum to 1, so the V bias contributes exactly b_vs to o;
  b' = b_vs @ w_fc + b_fc is folded into the residual on the host:
  xqT carries (x + b')^T and b_qs is corrected by -b' @ w_qs.
- 1/TEMP is folded into w_qs/b_qs on the host.

Pipeline per core (feature-major activations, transposed on the HOST):
- QT/KT feature-major via matmul(lhsT=weight tile, rhs=xT), V token-major.
- attention in 11 chunks of 96 tokens, head PAIRS: band mask preloaded
  into PSUM by a PE matmul (identity x mask, start=True), score matmuls
  accumulate on top (start=False); ACT Exp reads PSUM directly -> bf16
  probs in SBUF + per-query row sums via accum_out; DVE reciprocal; the
  prob transpose is a matmul by diag(1/Z) (normalization fused into the
  transpose); PV matmuls accumulate both heads into one (128, 96) PSUM.
- FC consumes OT; the residual x rides the same PSUM accumulation via
  identity matmuls from xT; LayerNorm runs directly off PSUM (ACT Square
  accum for sumsq, DVE reduce for sum, ACT Identity scale/bias evict).
- V projection / attention chunks / FC+LN are interleaved in emission
  order to keep the PE instruction stream dense (p-state at 2.4 GHz).
"""

import sys

for _p in ("/opt/trn_rl_repo",):
    if _p not in sys.path:
        sys.path.insert(0, _p)

import numpy as np
import ml_dtypes

BF16 = ml_dtypes.bfloat16

H, DK, DV, D = 16, 64, 64, 1024
NEI = 3
TEMP = 8.0
EPS = 1e-5
B, S = 4, 2048
NCORES = 8
T = (B * S) // NCORES          # 1024 tokens per core
TH = T + 2 * NEI               # 1030 halo tokens
P = 128
NT = T // P                    # 8 fc-phase token chunks
ND = D // P                    # 8 feature chunks
CL = 96                        # attention chunk length
CST = [96 * i for i in range(10)] + [928]          # chunk starts
NCH = len(CST)                 # 11 attention chunks
TH2 = 1056                     # padded halo width (window reads up to 1056)
NEG = -30000.0

_CACHE = {}


def _build_program(apply_affine: bool):
    import concourse.bacc as bacc
    import concourse.tile as tile
    from concourse import mybir
    from contextlib import ExitStack

    # Pin all activations to the one table set that contains every function
    # we use (Exp, Ln, Identity, Copy, Square). The default first-fit pass
    # alternates between exp- and ln- sets, costing a 1.3us table reload per
    # FC chunk. Indices into act_info.json are preserved; only the candidate
    # function sets are restricted. Restored after compile.
    _orig_tables = bacc.get_activation_tables

    def _pinned_tables(arch):
        t = _orig_tables(arch)
        keep = "natural_log_exp_and_others"
        if keep not in t:
            return t
        return {name: (funcs if name == keep else frozenset())
                for name, funcs in t.items()}

    f32 = mybir.dt.float32
    bf16 = mybir.dt.bfloat16
    Alu = mybir.AluOpType
    Act = mybir.ActivationFunctionType

    bacc.get_activation_tables = _pinned_tables
    nc = bacc.Bacc(
        "TRN2", target_bir_lowering=False, debug=False, enable_asserts=False
    )

    def din(name, shape, dt_):
        return nc.dram_tensor(name, shape, dt_, kind="ExternalInput").ap()

    xqT = din("xqT", (D, T), bf16)       # (x + b')^T (host-transposed)
    xsT = din("xsT", (D, TH), bf16)      # xs^T with halo (host-transposed)
    wq = din("wq", (D, D), bf16)         # w_qs / TEMP
    wk = din("wk", (D, D), bf16)
    wv = din("wv", (D, D), bf16)
    wf = din("wf", (D, D), bf16)
    bq = din("bq", (P, ND), f32)         # (b_qs - b'@w_qs)/TEMP laid out [p, ec]
    msk = din("msk", (CL, P), bf16)      # band mask 0 / NEG
    id96 = din("id96", (CL, CL), bf16)   # identity for mask preload + transpose
    id128 = din("id128", (P, P), bf16)   # identity for residual matmul
    if apply_affine:
        lng = din("lng", (1, D), f32)
        lnb = din("lnb", (1, D), f32)
    yo = nc.dram_tensor("yo", (T, D), f32, kind="ExternalOutput").ap()

    with tile.TileContext(nc) as tc, ExitStack() as ctx:
        consts = ctx.enter_context(tc.tile_pool(name="consts", bufs=1))
        big = ctx.enter_context(tc.tile_pool(name="big", bufs=1))
        wpool = ctx.enter_context(tc.tile_pool(name="wpool", bufs=2))
        work = ctx.enter_context(tc.tile_pool(name="work", bufs=3))
        opool = ctx.enter_context(tc.tile_pool(name="opool", bufs=2))
        small = ctx.enter_context(tc.tile_pool(name="small", bufs=4))
        ps1 = ctx.enter_context(tc.tile_pool(name="ps1", bufs=3, space="PSUM"))
        psT = ctx.enter_context(tc.tile_pool(name="psT", bufs=1, space="PSUM"))
        psO = ctx.enter_context(tc.tile_pool(name="psO", bufs=1, space="PSUM"))
        psF = ctx.enter_context(tc.tile_pool(name="psF", bufs=1, space="PSUM"))

        # ---- weights first (Q-proj critical path), then consts ----
        def load_w(wap, tagp):
            tiles = []
            for dc in range(ND):
                wt = wpool.tile([P, D], bf16, tag=f"w{dc}", name=f"w_{tagp}{dc}")
                nc.sync.dma_start(out=wt, in_=wap[dc * P:(dc + 1) * P, :])
                tiles.append(wt)
            return tiles

        wq_t = load_w(wq, "q")
        xT_t = []
        for dc in range(ND):
            t1 = big.tile([P, T], bf16, tag=f"xT{dc}", name=f"xT{dc}")
            nc.sync.dma_start(out=t1, in_=xqT[dc * P:(dc + 1) * P, :])
            xT_t.append(t1)
        wk_t = load_w(wk, "k")
        xsT_t = []
        for dc in range(ND):
            t2 = big.tile([P, TH2], bf16, tag=f"xsT{dc}", name=f"xsT{dc}")
            nc.sync.dma_start(out=t2[:, 0:TH], in_=xsT[dc * P:(dc + 1) * P, :])
            nc.vector.memset(t2[:, TH:TH2], 0.0)
            xsT_t.append(t2)

        bq_sb = consts.tile([P, ND], f32, tag="bq")
        nc.sync.dma_start(out=bq_sb, in_=bq)
        msk_sb = consts.tile([CL, P], bf16, tag="msk")
        nc.sync.dma_start(out=msk_sb, in_=msk)
        id96_sb = consts.tile([CL, CL], bf16, tag="id96")
        nc.sync.dma_start(out=id96_sb, in_=id96)
        id128_sb = consts.tile([P, P], bf16, tag="id128")
        nc.sync.dma_start(out=id128_sb, in_=id128)
        eps_sb = consts.tile([P, 1], f32, tag="eps")
        nc.vector.memset(eps_sb, EPS)
        if apply_affine:
            import concourse.bass as bass

            g_bc = consts.tile([P, D], f32, tag="g_bc")
            b_bc = consts.tile([P, D], f32, tag="b_bc")
            nc.sync.dma_start(
                out=g_bc,
                in_=bass.AP(tensor=lng.tensor, offset=lng.offset,
                            ap=[[0, P]] + list(lng.ap[1:])),
            )
            nc.sync.dma_start(
                out=b_bc,
                in_=bass.AP(tensor=lnb.tensor, offset=lnb.offset,
                            ap=[[0, P]] + list(lnb.ap[1:])),
            )

        # ---- QT projection: (e, t) feature-major, bias via ACT evict ----
        QT = [big.tile([P, T], bf16, tag=f"QT{ec}", name=f"QT{ec}")
              for ec in range(ND)]
        for ec in range(ND):
            psa = ps1.tile([P, 512], f32, tag="ps1", name="ps_qa")
            psb = ps1.tile([P, 512], f32, tag="ps1", name="ps_qb")
            for dc in range(ND):
                lt = wq_t[dc][:, ec * P:(ec + 1) * P]
                nc.tensor.matmul(psa, lhsT=lt, rhs=xT_t[dc][:, 0:512],
                                 start=(dc == 0), stop=(dc == ND - 1))
                nc.tensor.matmul(psb, lhsT=lt, rhs=xT_t[dc][:, 512:1024],
                                 start=(dc == 0), stop=(dc == ND - 1))
            nc.scalar.activation(out=QT[ec][:, 0:512], in_=psa,
                                 func=Act.Identity,
                                 bias=bq_sb[:, ec:ec + 1], scale=1.0)
            nc.scalar.activation(out=QT[ec][:, 512:1024], in_=psb,
                                 func=Act.Identity,
                                 bias=bq_sb[:, ec:ec + 1], scale=1.0)

        # ---- KT projection: (e, t_halo) feature-major, no bias ----
        KT = [big.tile([P, TH2], bf16, tag=f"KT{ec}", name=f"KT{ec}")
              for ec in range(ND)]
        for ec in range(ND):
            psa = ps1.tile([P, 512], f32, tag="ps1", name="ps_ka")
            psb = ps1.tile([P, 512], f32, tag="ps1", name="ps_kb")
            for dc in range(ND):
                lt = wk_t[dc][:, ec * P:(ec + 1) * P]
                nc.tensor.matmul(psa, lhsT=lt, rhs=xsT_t[dc][:, 0:512],
                                 start=(dc == 0), stop=(dc == ND - 1))
                nc.tensor.matmul(psb, lhsT=lt, rhs=xsT_t[dc][:, 512:1024],
                                 start=(dc == 0), stop=(dc == ND - 1))
            nc.vector.tensor_copy(KT[ec][:, 0:512], psa)
            nc.vector.tensor_copy(KT[ec][:, 512:1024], psb)
        for ec in range(ND):  # halo tail (incl zero padding)
            pst = ps1.tile([P, 512], f32, tag="ps1", name="ps_kt")
            for dc in range(ND):
                nc.tensor.matmul(pst[:, 0:TH2 - T],
                                 lhsT=wk_t[dc][:, ec * P:(ec + 1) * P],
                                 rhs=xsT_t[dc][:, T:TH2],
                                 start=(dc == 0), stop=(dc == ND - 1))
            nc.vector.tensor_copy(KT[ec][:, T:TH2], pst[:, 0:TH2 - T])

        # weight prefetch for the fused phase
        wv_t = load_w(wv, "v")
        wf_t = load_w(wf, "f")

        V = [None] * NCH
        OT = [big.tile([P, T], bf16, tag=f"OT{ec}", name=f"OT{ec}")
              for ec in range(ND)]

        # ---------- fused-phase emission helpers ----------
        def emit_v(ci):
            s = CST[ci]
            vt = big.tile([P, D], bf16, tag=f"V{ci}", name=f"V{ci}")
            psa = ps1.tile([P, 512], f32, tag="ps1", name="ps_va")
            psb = ps1.tile([P, 512], f32, tag="ps1", name="ps_vb")
            for dc in range(ND):
                lt = xsT_t[dc][:, s:s + P]
                nc.tensor.matmul(psa, lhsT=lt, rhs=wv_t[dc][:, 0:512],
                                 start=(dc == 0), stop=(dc == ND - 1))
                nc.tensor.matmul(psb, lhsT=lt, rhs=wv_t[dc][:, 512:1024],
                                 start=(dc == 0), stop=(dc == ND - 1))
            nc.vector.tensor_copy(vt[:, 0:512], psa)
            nc.vector.tensor_copy(vt[:, 512:1024], psb)
            V[ci] = vt

        # persistent normalized-prob tiles: live key cols [0:LW); the dead
        # tail [LW:P) is zeroed once so transposes can read a full 128 keys
        LW = 104  # live key window per 96-query chunk (96 + 6, padded to x8)
        pn_tiles = []
        for j in range(3):
            pnt = big.tile([CL, 2 * P], bf16, tag=f"pn{j}", name=f"pn{j}")
            nc.vector.memset(
                pnt.rearrange("p (h w) -> p h w", h=2)[:, :, LW:P], 0.0)
            pn_tiles.append(pnt)

        def pair_head(idx, ci, ec):
            s = CST[ci]
            # scores + band mask accumulated in two PSUM banks (one/head)
            sa = ps1.tile([P, 512], f32, tag="ps1", name="sa")
            sb = ps1.tile([P, 512], f32, tag="ps1", name="sb")
            nc.tensor.matmul(sa[0:CL, 0:LW], lhsT=id96_sb,
                             rhs=msk_sb[:, 0:LW], start=True, stop=False)
            nc.tensor.matmul(sa[0:CL, 0:LW],
                             lhsT=QT[ec][0:64, s:s + CL],
                             rhs=KT[ec][0:64, s:s + LW],
                             start=False, stop=True)
            nc.tensor.matmul(sb[0:CL, 0:LW], lhsT=id96_sb,
                             rhs=msk_sb[:, 0:LW], start=True, stop=False)
            nc.tensor.matmul(sb[0:CL, 0:LW],
                             lhsT=QT[ec][64:128, s:s + CL],
                             rhs=KT[ec][64:128, s:s + LW],
                             start=False, stop=True)
            return {"idx": idx, "ci": ci, "ec": ec, "sa": sa, "sb": sb}

        def pair_mid(st):
            pe2 = work.tile([CL, 2 * LW], bf16, tag="pe2", name="pe2")
            nc.scalar.activation(out=pe2[:, 0:LW],
                                 in_=st["sa"][0:CL, 0:LW], func=Act.Exp)
            nc.scalar.activation(out=pe2[:, LW:2 * LW],
                                 in_=st["sb"][0:CL, 0:LW], func=Act.Exp)
            rs2 = small.tile([CL, 2], f32, tag="rs2", name="rs2")
            nc.vector.tensor_reduce(
                out=rs2, in_=pe2.rearrange("p (h w) -> p h w", h=2),
                axis=mybir.AxisListType.X, op=Alu.add)
            rsr2 = small.tile([CL, 2], f32, tag="rsr2", name="rsr2")
            nc.vector.reciprocal(rsr2, rs2)
            pn2 = pn_tiles[st["idx"] % 3]
            nc.gpsimd.tensor_tensor(
                pn2.rearrange("p (h w) -> p h w", h=2)[:, :, 0:LW],
                pe2.rearrange("p (h w) -> p h w", h=2),
                rsr2[:, :, None].to_broadcast((CL, 2, LW)),
                Alu.mult,
            )
            st["pn2"] = pn2

        def pair_tail(st):
            ci, ec, pn2 = st["ci"], st["ec"], st["pn2"]
            s = CST[ci]
            pt = psT.tile([P, 2048], bf16, tag="psT", name="pt")
            nc.tensor.transpose(pt[:, 0:CL], pn2[:, 0:P], id96_sb)
            nc.tensor.transpose(pt[:, 1024:1024 + CL], pn2[:, P:2 * P],
                                id96_sb)
            ptsb = work.tile([P, 2 * CL], bf16, tag="ptsb", name="ptsb")
            nc.vector.tensor_copy(
                ptsb.rearrange("p (h w) -> p h w", h=2),
                pt.rearrange("p (h w) -> p h w", h=2)[:, 0:2, 0:CL],
            )
            ot2 = psO.tile([P, 512], f32, tag="psO", name="ot2")
            nc.tensor.matmul(ot2[0:64, 0:CL],
                             lhsT=V[ci][:, ec * P:ec * P + 64],
                             rhs=ptsb[:, 0:CL], start=True, stop=True)
            nc.tensor.matmul(ot2[64:128, 0:CL],
                             lhsT=V[ci][:, ec * P + 64:(ec + 1) * P],
                             rhs=ptsb[:, CL:2 * CL], start=True, stop=True)
            nc.scalar.activation(out=OT[ec][:, s:s + CL], in_=ot2[:, 0:CL],
                                 func=Act.Copy)

        def emit_fc(c):
            cs = slice(c * P, (c + 1) * P)
            fa = psF.tile([P, 512], f32, tag="fA", name="fa")
            fb = psF.tile([P, 512], f32, tag="fB", name="fb")
            # one full-region start per bank; residual (x + b') identity
            # matmuls accumulate into 128-col subregions of the open group
            nc.tensor.matmul(fa, lhsT=OT[0][:, cs], rhs=wf_t[0][:, 0:512],
                             start=True, stop=False)
            nc.tensor.matmul(fb, lhsT=OT[0][:, cs], rhs=wf_t[0][:, 512:1024],
                             start=True, stop=False)
            for db in range(4):
                nc.tensor.matmul(fa[:, db * P:(db + 1) * P],
                                 lhsT=xT_t[db][:, cs], rhs=id128_sb,
                                 start=False, stop=False)
                nc.tensor.matmul(fb[:, db * P:(db + 1) * P],
                                 lhsT=xT_t[4 + db][:, cs], rhs=id128_sb,
                                 start=False, stop=False)
            for ec in range(1, ND):
                lt = OT[ec][:, cs]
                nc.tensor.matmul(fa, lhsT=lt, rhs=wf_t[ec][:, 0:512],
                                 start=False, stop=(ec == ND - 1))
                nc.tensor.matmul(fb, lhsT=lt, rhs=wf_t[ec][:, 512:1024],
                                 start=False, stop=(ec == ND - 1))
            # evict y to SBUF bf16 IMMEDIATELY to release the psF banks
            # (keeping LN reads on PSUM stalls the next chunk's FC matmuls)
            y_sb = opool.tile([P, D], bf16, tag="ysb", name="y_sb")
            ysA = small.tile([P, 1], f32, tag="ysA", name="ysA")
            ysB = small.tile([P, 1], f32, tag="ysB", name="ysB")
            nc.scalar.activation(out=y_sb[:, 0:512], in_=fa, func=Act.Copy)
            nc.vector.tensor_copy(y_sb[:, 512:1024], fb)
            nc.vector.tensor_reduce(out=ysA, in_=y_sb[:, 0:512],
                                    axis=mybir.AxisListType.X, op=Alu.add)
            nc.vector.tensor_reduce(out=ysB, in_=y_sb[:, 512:1024],
                                    axis=mybir.AxisListType.X, op=Alu.add)
            ysq = opool.tile([P, D], bf16, tag="ysq", name="ysq")
            sqA = small.tile([P, 1], f32, tag="sqA", name="sqA")
            sqB = small.tile([P, 1], f32, tag="sqB", name="sqB")
            nc.vector.tensor_mul(ysq[:, 0:512], y_sb[:, 0:512],
                                 y_sb[:, 0:512])
            nc.vector.tensor_mul(ysq[:, 512:1024], y_sb[:, 512:1024],
                                 y_sb[:, 512:1024])
            nc.vector.tensor_reduce(out=sqA, in_=ysq[:, 0:512],
                                    axis=mybir.AxisListType.X, op=Alu.add)
            nc.vector.tensor_reduce(out=sqB, in_=ysq[:, 512:1024],
                                    axis=mybir.AxisListType.X, op=Alu.add)
            ysum = small.tile([P, 1], f32, tag="ysum", name="ysum")
            nc.vector.tensor_add(ysum, ysA, ysB)
            ssum = small.tile([P, 1], f32, tag="ssum", name="ssum")
            nc.vector.tensor_add(ssum, sqA, sqB)
            mean = small.tile([P, 1], f32, tag="mean", name="mean")
            nc.vector.tensor_scalar_mul(mean, ysum, 1.0 / D)
            msq = small.tile([P, 1], f32, tag="msq", name="msq")
            nc.vector.tensor_mul(msq, mean, mean)
            var = small.tile([P, 1], f32, tag="var", name="var")
            nc.vector.scalar_tensor_tensor(
                out=var, in0=ssum, scalar=1.0 / D, in1=msq,
                op0=Alu.mult, op1=Alu.subtract,
            )
            # rstd = exp(-0.5*ln(var+eps)): keeps ACT on one function table
            # (Sqrt lives in a different act set and forces 1.3us reloads)
            lnv = small.tile([P, 1], f32, tag="lnv", name="lnv")
            nc.scalar.activation(out=lnv, in_=var, func=Act.Ln, bias=eps_sb)
            rstd = small.tile([P, 1], f32, tag="rstd", name="rstd")
            nc.scalar.activation(out=rstd, in_=lnv, func=Act.Exp, scale=-0.5)
            bact = small.tile([P, 1], f32, tag="bact", name="bact")
            nc.vector.scalar_tensor_tensor(
                out=bact, in0=mean, scalar=-1.0, in1=rstd,
                op0=Alu.mult, op1=Alu.mult,
            )
            out_sb = opool.tile([P, D], f32, tag="osb", name="out_sb")
            nc.scalar.activation(out=out_sb[:, 0:512], in_=y_sb[:, 0:512],
                                 func=Act.Identity, bias=bact, scale=rstd)
            nc.scalar.activation(out=out_sb[:, 512:1024],
                                 in_=y_sb[:, 512:1024],
                                 func=Act.Identity, bias=bact, scale=rstd)
            if apply_affine:
                nc.vector.tensor_mul(out_sb, out_sb, g_bc)
                nc.vector.tensor_add(out_sb, out_sb, b_bc)
            nc.sync.dma_start(out=yo[cs, :], in_=out_sb)

        # FC chunk c is ready after attention chunk a_last(c)
        a_last = [((c + 1) * P - 1) // CL for c in range(NT)]
        a_last[NT - 1] = NCH - 1

        # ---------- fused phase: V / attention / FC interleaved ----------
        # 3-stage software pipeline over head pairs: emit pair i's scores
        # (PE) two pairs ahead of its transpose/PV (PE), so the PE stream
        # never head-of-line blocks on the cross-engine softmax chain.
        emit_v(0)
        emit_v(1)
        seq = [(ci, ec) for ci in range(NCH) for ec in range(ND)]
        stq = []
        done = 0

        def after_tail(st):
            # FC chunks become ready once the last pair of chunk ci retires
            if st["ec"] == ND - 1:
                for c in range(NT):
                    if a_last[c] == st["ci"]:
                        emit_fc(c)

        for idx, (ci, ec) in enumerate(seq):
            stq.append(pair_head(idx, ci, ec))
            if ec == 3 and ci + 2 < NCH:
                emit_v(ci + 2)
            if len(stq) - done >= 2:
                pair_mid(stq[-2])
            if len(stq) - done >= 3:
                pair_tail(stq[done])
                after_tail(stq[done])
                stq[done] = None
                done += 1
        pair_mid(stq[-1])
        for k in range(done, len(seq)):
            pair_tail(stq[k])
            after_tail(stq[k])
            stq[k] = None

    try:
        nc.compile()
    finally:
        bacc.get_activation_tables = _orig_tables
    return nc


def _get_program(apply_affine: bool):
    key = ("prog", apply_affine)
    if key not in _CACHE:
        _CACHE[key] = _build_program(apply_affine)
    return _CACHE[key]


def _host_prep(inputs):
    x = np.asarray(inputs["x"], np.float32)
    xs = np.asarray(inputs["xs"], np.float32)
    w_qs = np.asarray(inputs["w_qs"], np.float32)
    b_qs = np.asarray(inputs["b_qs"], np.float32)
    w_ks = np.asarray(inputs["w_ks"], np.float32)
    w_vs = np.asarray(inputs["w_vs"], np.float32)
    b_vs = np.asarray(inputs["b_vs"], np.float32)
    w_fc = np.asarray(inputs["w_fc"], np.float32)
    b_fc = np.asarray(inputs["b_fc"], np.float32)
    ln_g = np.asarray(inputs["ln_g"], np.float32)
    ln_b = np.asarray(inputs["ln_b"], np.float32)

    apply_affine = not (np.all(ln_g == 1.0) and np.all(ln_b == 0.0))

    bprime = (b_vs @ w_fc + b_fc).astype(np.float32)
    bq_eff = (b_qs - bprime @ w_qs) / TEMP

    mask = np.full((CL, P), NEG, np.float32)
    for t in range(CL):
        mask[t, t:t + 2 * NEI + 1] = 0.0

    shared = {
        "wq": np.ascontiguousarray((w_qs / TEMP).astype(BF16)),
        "wk": np.ascontiguousarray(w_ks.astype(BF16)),
        "wv": np.ascontiguousarray(w_vs.astype(BF16)),
        "wf": np.ascontiguousarray(w_fc.astype(BF16)),
        "bq": np.ascontiguousarray(bq_eff.reshape(ND, P).T.astype(np.float32)),
        "msk": np.ascontiguousarray(mask.astype(BF16)),
        "id96": np.eye(CL, dtype=BF16),
        "id128": np.eye(P, dtype=BF16),
    }
    if apply_affine:
        shared["lng"] = np.ascontiguousarray(ln_g.reshape(1, D))
        shared["lnb"] = np.ascontiguousarray(ln_b.reshape(1, D))

    xr = x + bprime[None, None, :]
    in_maps = []
    half_n = S // 2  # 1024
    for core in range(NCORES):
        b, half = core // 2, core % 2
        t0 = half * half_n
        halo = np.zeros((TH, D), np.float32)
        lo = max(0, t0 - NEI)
        hi = min(S, t0 + half_n + NEI)
        halo[lo - (t0 - NEI):hi - (t0 - NEI)] = xs[b, lo:hi]
        m = dict(shared)
        m["xqT"] = np.ascontiguousarray(xr[b, t0:t0 + half_n].T.astype(BF16))
        m["xsT"] = np.ascontiguousarray(halo.T.astype(BF16))
        in_maps.append(m)
    return in_maps, apply_affine


def _run(inputs, trace=False, trace_kwargs=None):
    from concourse.bass_utils import run_bass_kernel_spmd

    in_maps, apply_affine = _host_prep(inputs)
    nc = _get_program(apply_affine)
    res = run_bass_kernel_spmd(
        nc, in_maps, list(range(NCORES)),
        trace=trace, **(trace_kwargs or {})
    )
    y = np.empty((B, S, D), np.float32)
    half_n = S // 2
    for core in range(NCORES):
        b, half = core // 2, core % 2
        y[b, half * half_n:(half + 1) * half_n] = res.results[core]["yo"]
    return y, res


def kernel(**inputs):
    y, _ = _run(inputs)
    return y
